# revision 2
# baseline (speedup 1.0000x reference)
"""MoE routing kernel (nn_EnhancedBrain) for Trainium2, 8 NeuronCores.

Strategy
--------
The router (mean-pool -> tiny MLP -> softmax -> top-3 -> renormalize) costs
~8 MFLOP vs ~1.7 TFLOP for the expert MLPs, and its only effect on the math
is which 3 of the 8 zone weights are nonzero per batch row.  It runs on host
in float64; the device computes exactly the nonzero-weight expert MLPs.

Sharding: by batch row.  Core c gets rows 2c and 2c+1, each with its 3
selected experts -> 6 equal expert-MLP passes per core, perfectly balanced,
no collectives.  The renormalized top-k weight is folded into that pair's Wb
copy on host.

Device kernel (per core), all fp32 storage with fp32r matmuls:
  for each batch row, for each half of the 2048 tokens (SBUF fit):
    z^T[d, t]  = x^T[d, t]                       (residual init, DVE copy)
    for each of the 3 experts, over f-chunks of 512:
      h^T[f, t] = gelu_tanh(Wa^T x^T)            (PE + ACT, PSUM chain over d)
      z^T      += (Wb * w)^T h^T                 (PE, PSUM chain over f; DVE add)
    y^T[d, t]  = z^T
Everything stays in the transposed [feature, token] layout so neither layer
needs a transpose; the host hands x^T in and transposes y^T back on gather.
"""

import numpy as np

import concourse.bass as bass
import concourse.mybir as mybir
import concourse.tile as tile
from concourse import bacc
from concourse.bass_utils import run_bass_kernel_spmd

B, S, D, F = 16, 2048, 1024, 4096
NZONES, TOPK = 8, 3
NCORES = 8
NB = B // NCORES            # batch rows per core = 2
NP = NB * TOPK              # (row, expert) pairs per core = 6
TH = 2                      # token halves per row
T = S // TH                 # tokens per half = 1024
TB = 512                    # matmul moving-dim block (fp32r needs >=256)
FC = 512                    # f-chunk held in SBUF at once
P = 128

F32 = mybir.dt.float32
F32R = mybir.dt.float32r
GELU = mybir.ActivationFunctionType.Gelu_apprx_tanh

_compiled_nc = None


def _build_nc(reps=1):
    from contextlib import nullcontext

    nc = bacc.Bacc("TRN2", target_bir_lowering=False)
    xt = nc.dram_tensor("xt", [NB, D, S], F32, kind="ExternalInput")
    wa = nc.dram_tensor("wa", [NP, D, F], F32, kind="ExternalInput")
    wb = nc.dram_tensor("wb", [NP, F, D], F32, kind="ExternalInput")
    y = nc.dram_tensor("y", [NB, D, S], F32, kind="ExternalOutput")

    nd, nf, nfc, ntb = D // P, FC // P, F // FC, T // TB

    with tile.TileContext(nc) as tc:
        with (
            tc.tile_pool(name="xtp", bufs=nd + 4) as xt_pool,
            tc.tile_pool(name="zp", bufs=nd + 2) as z_pool,
            tc.tile_pool(name="hp", bufs=3 * nf) as h_pool,
            tc.tile_pool(name="wap", bufs=nd + 3) as wa_pool,
            tc.tile_pool(name="wbp", bufs=3 * nf) as wb_pool,
            tc.tile_pool(name="ps1", bufs=4, space="PSUM") as ps1,
            tc.tile_pool(name="ps2", bufs=4, space="PSUM") as ps2,
            tc.For_i(0, reps, 1) if reps > 1 else nullcontext(),
        ):
            for bi in range(NB):
                for hh in range(TH):
                    t0 = hh * T
                    xts, zts = [], []
                    for dc in range(nd):
                        xtile = xt_pool.tile([P, T], F32R, tag="xt")
                        nc.sync.dma_start(
                            xtile[:],
                            xt[bi, dc * P:(dc + 1) * P, t0:t0 + T].bitcast(F32R),
                        )
                        xts.append(xtile)
                    for dc in range(nd):
                        ztile = z_pool.tile([P, T], F32, tag="z")
                        nc.vector.tensor_copy(ztile[:], xts[dc][:].bitcast(F32))
                        zts.append(ztile)
                    def emit_l2(chunks):
                        # z^T += (Wb*w)^T h^T for a group of f-chunks in one
                        # PSUM chain (chunks may span expert boundaries; the
                        # accumulation is linear so that is exact)
                        hts = [t for c in chunks for t in c[0]]
                        wbts = [t for c in chunks for t in c[1]]
                        nch = len(hts)
                        for dc in range(nd):
                            for tb in range(ntb):
                                s = slice(tb * TB, (tb + 1) * TB)
                                pz = ps2.tile([P, TB], F32, tag="ps2")
                                for fi in range(nch):
                                    nc.tensor.matmul(
                                        pz[:],
                                        wbts[fi][:, dc * P:(dc + 1) * P],
                                        hts[fi][:, s],
                                        start=(fi == 0),
                                        stop=(fi == nch - 1),
                                    )
                                nc.vector.tensor_tensor(
                                    zts[dc][:, s],
                                    zts[dc][:, s],
                                    pz[:],
                                    op=mybir.AluOpType.add,
                                )

                    # Software-pipelined with skew 1: L1 of chunk c+1 is
                    # emitted before L2 of chunk c so the in-order PE covers
                    # the gelu tail of chunk c with chunk c+1's matmuls.
                    pending = []
                    for k in range(TOPK):
                        pr = bi * TOPK + k
                        for fc in range(nfc):
                            f0 = fc * FC
                            wats = []
                            for dc in range(nd):
                                wt = wa_pool.tile([P, FC], F32R, tag="wa")
                                nc.sync.dma_start(
                                    wt[:],
                                    wa[pr, dc * P:(dc + 1) * P, f0:f0 + FC]
                                    .bitcast(F32R),
                                )
                                wats.append(wt)
                            wbts = []
                            for fi in range(nf):
                                wt = wb_pool.tile([P, D], F32R, tag="wb")
                                nc.sync.dma_start(
                                    wt[:],
                                    wb[pr, f0 + fi * P:f0 + (fi + 1) * P, :]
                                    .bitcast(F32R),
                                )
                                wbts.append(wt)
                            hts = []
                            for fi in range(nf):
                                ht = h_pool.tile([P, T], F32R, tag="h")
                                for tb in range(ntb):
                                    s = slice(tb * TB, (tb + 1) * TB)
                                    ph = ps1.tile([P, TB], F32, tag="ps1")
                                    for dc in range(nd):
                                        nc.tensor.matmul(
                                            ph[:],
                                            wats[dc][:, fi * P:(fi + 1) * P],
                                            xts[dc][:, s],
                                            start=(dc == 0),
                                            stop=(dc == nd - 1),
                                        )
                                    nc.scalar.activation(ht[:, s], ph[:], GELU)
                                hts.append(ht)
                            pending.append((hts, wbts))
                            if len(pending) == 3:
                                emit_l2(pending[:2])
                                pending = pending[2:]
                    emit_l2(pending)
                    for dc in range(nd):
                        nc.sync.dma_start(
                            y[bi, dc * P:(dc + 1) * P, t0:t0 + T], zts[dc][:]
                        )
    nc.compile()
    return nc


def _route(x, W1, b1, W2, b2):
    """Host router in float64; reproduces jax.lax.top_k tie-breaking."""
    pooled = x.mean(axis=1, dtype=np.float64)
    h = np.tanh(pooled @ W1.astype(np.float64) + b1.astype(np.float64))
    logits = h @ W2.astype(np.float64) + b2.astype(np.float64)
    e = np.exp(logits - logits.max(axis=-1, keepdims=True))
    probs = e / e.sum(axis=-1, keepdims=True)
    top_i = np.argsort(-probs, axis=-1, kind="stable")[:, :TOPK]
    top_p = np.take_along_axis(probs, top_i, axis=-1)
    top_w = top_p / top_p.sum(axis=-1, keepdims=True)
    return top_i, top_w


def make_in_maps(x, W1, b1, W2, b2, Wa, Wb):
    x = np.ascontiguousarray(np.asarray(x, dtype=np.float32))
    Wa = np.asarray(Wa, dtype=np.float32)
    Wb = np.asarray(Wb, dtype=np.float32)

    top_i, top_w = _route(
        x, np.asarray(W1), np.asarray(b1), np.asarray(W2), np.asarray(b2)
    )

    in_maps = []
    for c in range(NCORES):
        rows = [NB * c + i for i in range(NB)]
        xt = np.ascontiguousarray(x[rows].transpose(0, 2, 1))
        wa_l, wb_l = [], []
        for i, b in enumerate(rows):
            for k in range(TOPK):
                e = int(top_i[b, k])
                wa_l.append(Wa[e])
                wb_l.append(Wb[e] * np.float32(top_w[b, k]))
        in_maps.append({
            "xt": xt,
            "wa": np.ascontiguousarray(np.stack(wa_l)),
            "wb": np.ascontiguousarray(np.stack(wb_l)),
        })
    return in_maps


def kernel(x, W1, b1, W2, b2, Wa, Wb):
    global _compiled_nc
    if _compiled_nc is None:
        _compiled_nc = _build_nc()
    nc = _compiled_nc

    in_maps = make_in_maps(x, W1, b1, W2, b2, Wa, Wb)
    res = run_bass_kernel_spmd(nc, in_maps, core_ids=list(range(NCORES)))

    y = np.empty((B, S, D), dtype=np.float32)
    for c in range(NCORES):
        yt = res.results[c]["y"]                      # [NB, D, S]
        for i in range(NB):
            y[NB * c + i] = yt[i].T
    return y



# revision 5
# speedup vs baseline: 1.1268x; 1.1268x over previous
"""MoE routing kernel (nn_EnhancedBrain) for Trainium2, 8 NeuronCores.

Strategy
--------
The router (mean-pool -> tiny MLP -> softmax -> top-3 -> renormalize) costs
~8 MFLOP vs ~1.7 TFLOP for the expert MLPs, and its only effect on the math
is which 3 of the 8 zone weights are nonzero per batch row.  It runs on host
in float64; the device computes exactly the nonzero-weight expert MLPs.

Sharding: by batch row.  Core c gets rows 2c and 2c+1, each with its 3
selected experts -> 6 equal expert-MLP passes per core, perfectly balanced,
no collectives.  The renormalized top-k weight is folded into that pair's Wb
copy on host.

Precision: all matmul operands are bf16 (x, Wa, Wb*w cast on host), fp32
PSUM accumulation, fp32 z/residual.  End-to-end rel err ~2e-3 vs the fp32
reference (gate is 2e-2).  bf16 enables FWL weight loads (64 cyc vs 128 for
fp32r), and each stationary weight tile is reused for 2 consecutive matmuls
(the two 512-token halves), halving LDWEIGHTS cost again.

Device kernel (per core), transposed [feature, token] layout throughout:
  for each batch row, for each half of the 2048 tokens:
    z^T[d, t]  = x^T[d, t]                       (bf16 -> f32 copy)
    for each of the 3 experts, over f-chunks of 512:
      h^T[f, t] = gelu_tanh(Wa^T x^T)            (PE + ACT; 2 parallel PSUM
                                                  chains over d, tb-paired)
      z^T      += (Wb * w)^T h^T                 (PE; 2 parallel PSUM chains
                                                  over f, tb-paired; DVE add)
    y^T[d, t]  = z^T
"""

import numpy as np
import ml_dtypes

import concourse.bass as bass
import concourse.mybir as mybir
import concourse.tile as tile
from concourse import bacc
from concourse.bass_utils import run_bass_kernel_spmd

B, S, D, F = 16, 2048, 1024, 4096
NZONES, TOPK = 8, 3
NCORES = 8
NB = B // NCORES            # batch rows per core = 2
NP = NB * TOPK              # (row, expert) pairs per core = 6
TH = 2                      # token halves per row
T = S // TH                 # tokens per half = 1024
TB = 512                    # matmul moving-dim block (PSUM bank limit)
FC = 512                    # f-chunk held in SBUF at once
P = 128

F32 = mybir.dt.float32
BF16 = mybir.dt.bfloat16
NPBF16 = ml_dtypes.bfloat16
GELU = mybir.ActivationFunctionType.Gelu_apprx_tanh

_compiled_nc = None


def _build_nc(reps=1):
    from contextlib import nullcontext

    nc = bacc.Bacc("TRN2", target_bir_lowering=False)
    xt = nc.dram_tensor("xt", [NB, D, S], BF16, kind="ExternalInput")
    wa = nc.dram_tensor("wa", [NP, D, F], BF16, kind="ExternalInput")
    wb = nc.dram_tensor("wb", [NP, F, D], BF16, kind="ExternalInput")
    y = nc.dram_tensor("y", [NB, D, S], F32, kind="ExternalOutput")

    nd, nf, nfc, ntb = D // P, FC // P, F // FC, T // TB

    with tile.TileContext(nc) as tc:
        with (
            tc.tile_pool(name="xtp", bufs=nd + 2) as xt_pool,
            tc.tile_pool(name="zp", bufs=nd + 2) as z_pool,
            tc.tile_pool(name="hp", bufs=3 * nf) as h_pool,
            tc.tile_pool(name="wap", bufs=3 * nd) as wa_pool,
            tc.tile_pool(name="wbp", bufs=3 * nf) as wb_pool,
            tc.tile_pool(name="ps1", bufs=4, space="PSUM") as ps1,
            tc.tile_pool(name="ps2", bufs=4, space="PSUM") as ps2,
            tc.For_i(0, reps, 1) if reps > 1 else nullcontext(),
        ):
            for bi in range(NB):
                for hh in range(TH):
                    t0 = hh * T
                    xts, zts = [], []
                    for dc in range(nd):
                        xtile = xt_pool.tile([P, T], BF16, tag="xt")
                        nc.sync.dma_start(
                            xtile[:], xt[bi, dc * P:(dc + 1) * P, t0:t0 + T]
                        )
                        xts.append(xtile)
                    for dc in range(nd):
                        ztile = z_pool.tile([P, T], F32, tag="z")
                        nc.vector.tensor_copy(ztile[:], xts[dc][:])
                        zts.append(ztile)

                    def emit_l2(chunks):
                        # z^T += (Wb*w)^T h^T for a group of f-chunks in one
                        # PSUM chain per (dc, tb).  tb inner so each (fi, dc)
                        # stationary tile serves 2 consecutive matmuls.
                        hts = [t for c in chunks for t in c[0]]
                        wbts = [t for c in chunks for t in c[1]]
                        nch = len(hts)
                        for dc in range(nd):
                            pz = [ps2.tile([P, TB], F32, tag="ps2",
                                           name=f"pz{tb}")
                                  for tb in range(ntb)]
                            for fi in range(nch):
                                for tb in range(ntb):
                                    s = slice(tb * TB, (tb + 1) * TB)
                                    nc.tensor.matmul(
                                        pz[tb][:],
                                        wbts[fi][:, dc * P:(dc + 1) * P],
                                        hts[fi][:, s],
                                        start=(fi == 0),
                                        stop=(fi == nch - 1),
                                    )
                            for tb in range(ntb):
                                s = slice(tb * TB, (tb + 1) * TB)
                                nc.vector.tensor_tensor(
                                    zts[dc][:, s],
                                    zts[dc][:, s],
                                    pz[tb][:],
                                    op=mybir.AluOpType.add,
                                )

                    # Software-pipelined with skew 1: L1 of chunk c+1 is
                    # emitted before L2 of chunk c so the in-order PE covers
                    # the gelu tail of chunk c with chunk c+1's matmuls.
                    pending = []
                    for k in range(TOPK):
                        pr = bi * TOPK + k
                        for fc in range(nfc):
                            f0 = fc * FC
                            wats = []
                            for dc in range(nd):
                                wt = wa_pool.tile([P, FC], BF16, tag="wa")
                                nc.sync.dma_start(
                                    wt[:],
                                    wa[pr, dc * P:(dc + 1) * P, f0:f0 + FC],
                                )
                                wats.append(wt)
                            wbts = []
                            for fi in range(nf):
                                wt = wb_pool.tile([P, D], BF16, tag="wb")
                                nc.sync.dma_start(
                                    wt[:],
                                    wb[pr, f0 + fi * P:f0 + (fi + 1) * P, :],
                                )
                                wbts.append(wt)
                            hts = []
                            for fi in range(nf):
                                ht = h_pool.tile([P, T], BF16, tag="h")
                                ph = [ps1.tile([P, TB], F32, tag="ps1",
                                               name=f"ph{tb}")
                                      for tb in range(ntb)]
                                for dc in range(nd):
                                    for tb in range(ntb):
                                        s = slice(tb * TB, (tb + 1) * TB)
                                        nc.tensor.matmul(
                                            ph[tb][:],
                                            wats[dc][:, fi * P:(fi + 1) * P],
                                            xts[dc][:, s],
                                            start=(dc == 0),
                                            stop=(dc == nd - 1),
                                        )
                                for tb in range(ntb):
                                    s = slice(tb * TB, (tb + 1) * TB)
                                    nc.scalar.activation(
                                        ht[:, s], ph[tb][:], GELU
                                    )
                                hts.append(ht)
                            pending.append((hts, wbts))
                            if len(pending) == 3:
                                emit_l2(pending[:2])
                                pending = pending[2:]
                    emit_l2(pending)
                    for dc in range(nd):
                        nc.sync.dma_start(
                            y[bi, dc * P:(dc + 1) * P, t0:t0 + T], zts[dc][:]
                        )
    nc.compile()
    return nc


def _route(x, W1, b1, W2, b2):
    """Host router in float64; reproduces jax.lax.top_k tie-breaking."""
    pooled = x.mean(axis=1, dtype=np.float64)
    h = np.tanh(pooled @ W1.astype(np.float64) + b1.astype(np.float64))
    logits = h @ W2.astype(np.float64) + b2.astype(np.float64)
    e = np.exp(logits - logits.max(axis=-1, keepdims=True))
    probs = e / e.sum(axis=-1, keepdims=True)
    top_i = np.argsort(-probs, axis=-1, kind="stable")[:, :TOPK]
    top_p = np.take_along_axis(probs, top_i, axis=-1)
    top_w = top_p / top_p.sum(axis=-1, keepdims=True)
    return top_i, top_w


def make_in_maps(x, W1, b1, W2, b2, Wa, Wb):
    x = np.ascontiguousarray(np.asarray(x, dtype=np.float32))
    Wa = np.asarray(Wa, dtype=np.float32)
    Wb = np.asarray(Wb, dtype=np.float32)

    top_i, top_w = _route(
        x, np.asarray(W1), np.asarray(b1), np.asarray(W2), np.asarray(b2)
    )

    Wa_bf = Wa.astype(NPBF16)
    in_maps = []
    for c in range(NCORES):
        rows = [NB * c + i for i in range(NB)]
        xt = np.ascontiguousarray(
            x[rows].transpose(0, 2, 1).astype(NPBF16)
        )
        wa_l, wb_l = [], []
        for i, b in enumerate(rows):
            for k in range(TOPK):
                e = int(top_i[b, k])
                wa_l.append(Wa_bf[e])
                wb_l.append((Wb[e] * np.float32(top_w[b, k])).astype(NPBF16))
        in_maps.append({
            "xt": xt,
            "wa": np.ascontiguousarray(np.stack(wa_l)),
            "wb": np.ascontiguousarray(np.stack(wb_l)),
        })
    return in_maps


def kernel(x, W1, b1, W2, b2, Wa, Wb):
    global _compiled_nc
    if _compiled_nc is None:
        _compiled_nc = _build_nc()
    nc = _compiled_nc

    in_maps = make_in_maps(x, W1, b1, W2, b2, Wa, Wb)
    res = run_bass_kernel_spmd(nc, in_maps, core_ids=list(range(NCORES)))

    y = np.empty((B, S, D), dtype=np.float32)
    for c in range(NCORES):
        yt = res.results[c]["y"]                      # [NB, D, S]
        for i in range(NB):
            y[NB * c + i] = yt[i].T
    return y
